# revision 9
# baseline (speedup 1.0000x reference)
"""Trainium2 Bass kernel for nn_Attention_15676630631260 (sparse_attention).

reference:
  q = x @ Wq.T + bq ; k = x @ Wk.T + bk ; v = x @ Wv.T + bv        (per batch)
  scores = sigmoid(q @ k.T / sqrt(P))                               [B,S,S]
  out[b,i,j,:] = tril(i,j) * scores[b,i,j] * v[b,j,:]               [B,S,S,P]

B=2, S=512, D=256, P=128.  Output is 256 MB; the causal mask zeroes the
j>i region.  run_bass_kernel_spmd pre-zeroes ExternalOutput buffers
(donated zero buffers under PJRT), so the kernel only writes the j<=i
region — at 128-column tile granularity per row: row i writes j-tiles
0..i//128.

Sharding (8 cores, one NEFF, SPMD): core c -> batch b=c//4, quarter
k=c%4.  Rows are assigned as 16-row blocks paired (m, 31-m) so every
core's multiset of per-row written-tile-counts is {1,1,2,2,3,3,4,4} per
16-row block pair -> identical instruction stream on every core, only
input data differs, and every core writes exactly 20 MB.

Per-core device program:
  K^T[p,s], Q^T[p,i] per-partition-bias via K=1 matmul; V[s,p] tiles.
  scores^T[j,i] = sigmoid((K^T_tile)^T @ Q^T / sqrt(P)) ; masked by a
  host-supplied per-core mask (handles the causal diagonal exactly).
  Output rows are produced as [j_partition, (i,jt,p)] slabs via
  per-partition-scalar broadcast multiplies (DVE tensor_scalar + ACT
  activation-scale, interleaved), then batched HWDGE DMAs to DRAM.
"""

import os
import sys

import numpy as np

for _p in ("/root/.axon_site/_ro/trn_rl_repo", "/opt/trn_rl_repo"):
    if _p not in sys.path and os.path.isdir(_p):
        sys.path.append(_p)

import concourse.bass as bass
import concourse.mybir as mybir
from concourse.tile import TileContext
from concourse import bass_utils

F32 = mybir.dt.float32
B, S, D, P = 2, 512, 256, 128
NCORES = 8
GROUP = 8           # output rows per DMA group
NGROUPS = 128 // GROUP
INV_SQRT_P = float(1.0 / np.sqrt(np.float32(P)))


def _blocks16(k: int) -> list[int]:
    # 16-row blocks (32 per batch) for quarter k, ordered so written
    # j-tile count ti=m//8 ascends: [0,0,1,1,2,2,3,3]
    return [k, k + 4, k + 8, k + 12, 19 - k, 23 - k, 27 - k, 31 - k]


def _rows_sel(k: int) -> np.ndarray:
    return np.concatenate([np.arange(16 * m, 16 * m + 16) for m in _blocks16(k)])


def _build_nc() -> bass.Bass:
    nc = bass.Bass(trn_type="TRN2")

    xt = nc.dram_tensor("xt", [D, S], F32, kind="ExternalInput")       # x[b].T
    xq = nc.dram_tensor("xq", [D, 128], F32, kind="ExternalInput")     # x[b].T[:, rows]
    wq = nc.dram_tensor("wq", [D, P], F32, kind="ExternalInput")       # Wq.T
    wk = nc.dram_tensor("wk", [D, P], F32, kind="ExternalInput")
    wv = nc.dram_tensor("wv", [D, P], F32, kind="ExternalInput")
    bq = nc.dram_tensor("bq", [1, P], F32, kind="ExternalInput")
    bk = nc.dram_tensor("bk", [1, P], F32, kind="ExternalInput")
    bv = nc.dram_tensor("bv", [1, P], F32, kind="ExternalInput")
    mk = nc.dram_tensor("mk", [4, 128, 128], F32, kind="ExternalInput")
    # local output layout [j, i_local, p]: per-DMA-partition runs are
    # (i,p)-contiguous (4 KB per 8-row group) instead of 512 B
    out = nc.dram_tensor("out", [S, 128, P], F32, kind="ExternalOutput")

    with TileContext(nc) as tc:
        with (
            tc.tile_pool(name="const", bufs=1) as cpool,
            tc.tile_pool(name="psA", bufs=1, space="PSUM") as psA,
            tc.tile_pool(name="psB", bufs=2, space="PSUM") as psB,
            tc.tile_pool(name="slab", bufs=3) as spool,
        ):
            xt_sb = cpool.tile([128, 2 * S], F32, tag="xt")
            nc.sync.dma_start(
                xt_sb[:].rearrange("q (c s) -> q c s", c=2),
                xt.rearrange("(c p) s -> p c s", p=128),
            )
            xq_sb = cpool.tile([128, 2 * 128], F32, tag="xq")
            nc.sync.dma_start(
                xq_sb[:].rearrange("q (c m) -> q c m", c=2),
                xq.rearrange("(c p) m -> p c m", p=128),
            )
            w_sb = {}
            for name, w in (("wq", wq), ("wk", wk), ("wv", wv)):
                w_sb[name] = cpool.tile([128, 2 * P], F32, tag=name, name=f"w_{name}")
                nc.sync.dma_start(
                    w_sb[name][:].rearrange("q (c m) -> q c m", c=2),
                    w.rearrange("(c p) m -> p c m", p=128),
                )
            b_sb = {}
            for name, b in (("bq", bq), ("bk", bk), ("bv", bv)):
                b_sb[name] = cpool.tile([1, P], F32, tag=name, name=f"b_{name}")
                nc.sync.dma_start(b_sb[name][:], b[:])
            mk_sb = cpool.tile([128, 4 * 128], F32, tag="mk")
            nc.sync.dma_start(
                mk_sb[:].rearrange("q (t i) -> q t i", t=4),
                mk.rearrange("t j i -> j t i"),
            )

            ones_sb = cpool.tile([1, S], F32, tag="ones")
            nc.vector.memset(ones_sb[:], 1.0)

            # K^T [p, s] = Wk @ x.T + bk  (bias per partition via K=1 matmul)
            kt_ps = psA.tile([128, S], F32, tag="ktps")
            nc.tensor.matmul(kt_ps[:], w_sb["wk"][:, 0:128], xt_sb[:, 0:S], start=True, stop=False)
            nc.tensor.matmul(kt_ps[:], w_sb["wk"][:, 128:256], xt_sb[:, S : 2 * S], start=False, stop=False)
            nc.tensor.matmul(
                kt_ps[:], b_sb["bk"][0:1, :], ones_sb[0:1, 0:S], start=False, stop=True
            )
            kt_sb = cpool.tile([128, S], F32, tag="kt")
            nc.scalar.copy(kt_sb[:], kt_ps[:])

            # Q^T [p, i] over this core's 128 rows
            qt_ps = psA.tile([128, 128], F32, tag="qtps")
            nc.tensor.matmul(qt_ps[:], w_sb["wq"][:, 0:128], xq_sb[:, 0:128], start=True, stop=False)
            nc.tensor.matmul(qt_ps[:], w_sb["wq"][:, 128:256], xq_sb[:, 128:256], start=False, stop=False)
            nc.tensor.matmul(
                qt_ps[:], b_sb["bq"][0:1, :], ones_sb[0:1, 0:128], start=False, stop=True
            )
            qt_sb = cpool.tile([128, 128], F32, tag="qt")
            nc.scalar.copy(qt_sb[:], qt_ps[:])

            # V [s, p] tiles (s on partitions), bias via K=1 matmul
            v_sb = cpool.tile([128, 4 * P], F32, tag="v")
            for jt in range(4):
                v_ps = psB.tile([128, P], F32, tag="vps")
                nc.tensor.matmul(
                    v_ps[:],
                    xt_sb[:, jt * 128 : (jt + 1) * 128],
                    w_sb["wv"][:, 0:128],
                    start=True,
                    stop=False,
                )
                nc.tensor.matmul(
                    v_ps[:],
                    xt_sb[:, S + jt * 128 : S + (jt + 1) * 128],
                    w_sb["wv"][:, 128:256],
                    start=False,
                    stop=False,
                )
                nc.tensor.matmul(
                    v_ps[:], ones_sb[0:1, 0:128], b_sb["bv"][0:1, :], start=False, stop=True
                )
                nc.scalar.copy(v_sb[:, jt * P : (jt + 1) * P], v_ps[:])

            # scores^T [j, i] per j-tile: sigmoid(K_tile @ Q^T / sqrt(P)) * mask
            stm_sb = cpool.tile([128, 4 * 128], F32, tag="stm")
            for jt in range(4):
                s_ps = psB.tile([128, 128], F32, tag="sps")
                nc.tensor.matmul(
                    s_ps[:],
                    kt_sb[:, jt * 128 : (jt + 1) * 128],
                    qt_sb[:],
                    start=True,
                    stop=True,
                )
                st_sb = cpool.tile([128, 128], F32, tag=f"st{jt}")
                nc.scalar.activation(
                    st_sb[:],
                    s_ps[:],
                    mybir.ActivationFunctionType.Sigmoid,
                    scale=INV_SQRT_P,
                )
                nc.vector.tensor_mul(
                    stm_sb[:, jt * 128 : (jt + 1) * 128],
                    st_sb[:],
                    mk_sb[:, jt * 128 : (jt + 1) * 128],
                )

            # Output slabs: for each group of GROUP rows, L = written j-tiles.
            # slab layout [j_partition, (jt, i_local, p)]; DMA to
            # out[0:L*128 (as t,j), group rows, :] with j as partition dim.
            out_r = out.rearrange("(t j) i p -> j t (i p)", j=128)  # [128,4,16384]
            op_idx = 0
            for g in range(NGROUPS):
                L = g // (NGROUPS // 4) + 1
                slab = spool.tile([128, L * GROUP * 128], F32, tag=f"slab{L}")
                for ii in range(GROUP):
                    li = g * GROUP + ii
                    for jt in range(L):
                        dst = slab[:, (jt * GROUP + ii) * 128 : (jt * GROUP + ii + 1) * 128]
                        vsl = v_sb[:, jt * P : (jt + 1) * P]
                        ssc = stm_sb[:, jt * 128 + li : jt * 128 + li + 1]
                        if op_idx % 3 == 2:
                            nc.scalar.mul(dst, vsl, mul=ssc)
                        else:
                            nc.vector.tensor_scalar_mul(dst, vsl, ssc)
                        op_idx += 1
                nc.sync.dma_start(
                    out_r[:, 0:L, GROUP * 128 * g : GROUP * 128 * (g + 1)],
                    slab[:].rearrange("q (t ip) -> q t ip", t=L),
                )

    _split_multi_waits(nc)
    return nc


def _split_multi_waits(nc):
    """This toolchain's walrus accepts at most one sync wait per
    instruction; split extras into single-wait NoOps just before the
    instruction on the same engine queue (waits are ANDed preconditions,
    executed in order on the engine's queue — semantically identical)."""
    for fn in nc.m.functions:
        for blk in fn.blocks:
            insts = blk.instructions
            i = 0
            while i < len(insts):
                inst = insts[i]
                si = getattr(inst, "sync_info", None)
                if si is not None and si.on_wait is not None and len(si.on_wait) > 1:
                    waits = list(si.on_wait)
                    nops = [
                        mybir.InstNoOp(
                            name=nc.get_next_instruction_name(),
                            engine=inst.engine,
                            sync_info=mybir.SyncInfo(on_wait=[w], on_update=[]),
                            bass_nofuse=True,
                        )
                        for w in waits[:-1]
                    ]
                    si.on_wait = [waits[-1]]
                    insts[i:i] = nops
                    i += len(nops)
                i += 1


_NC_CACHE = None


def _get_nc():
    global _NC_CACHE
    if _NC_CACHE is None:
        _NC_CACHE = _build_nc()
    return _NC_CACHE


def _in_maps(x_set, Wq, bq, Wk, bk, Wv, bv):
    wqT = np.ascontiguousarray(Wq.T).astype(np.float32, copy=False)
    wkT = np.ascontiguousarray(Wk.T).astype(np.float32, copy=False)
    wvT = np.ascontiguousarray(Wv.T).astype(np.float32, copy=False)
    bq2 = np.ascontiguousarray(bq.reshape(1, P)).astype(np.float32, copy=False)
    bk2 = np.ascontiguousarray(bk.reshape(1, P)).astype(np.float32, copy=False)
    bv2 = np.ascontiguousarray(bv.reshape(1, P)).astype(np.float32, copy=False)
    xts = [np.ascontiguousarray(x_set[b].T).astype(np.float32, copy=False) for b in range(B)]
    jj = np.arange(128)
    maps = []
    for c in range(NCORES):
        b, k = divmod(c, 4)
        rows = _rows_sel(k)
        mask = np.empty((4, 128, 128), np.float32)
        for jt in range(4):
            mask[jt] = ((jt * 128 + jj)[:, None] <= rows[None, :]).astype(np.float32)
        maps.append(
            {
                "xt": xts[b],
                "xq": np.ascontiguousarray(xts[b][:, rows]),
                "wq": wqT,
                "wk": wkT,
                "wv": wvT,
                "bq": bq2,
                "bk": bk2,
                "bv": bv2,
                "mk": mask,
            }
        )
    return maps


def run(x_set, Wq, bq, Wk, bk, Wv, bv, **spmd_kwargs):
    nc = _get_nc()
    in_maps = _in_maps(x_set, Wq, bq, Wk, bk, Wv, bv)
    res = bass_utils.run_bass_kernel_spmd(
        nc, in_maps, core_ids=list(range(NCORES)), **spmd_kwargs
    )
    full = np.zeros((B, S, S, P), np.float32)
    for c in range(NCORES):
        b, k = divmod(c, 4)
        # core output is [j, i_local, p] -> scatter as [i_local, j, p]
        full[b, _rows_sel(k)] = res.results[c]["out"].transpose(1, 0, 2)
    return full, res


def kernel(x_set, Wq, bq, Wk, bk, Wv, bv):
    full, _ = run(x_set, Wq, bq, Wk, bk, Wv, bv)
    return full


# revision 11
# speedup vs baseline: 1.1750x; 1.1750x over previous
"""Trainium2 Bass kernel for nn_Attention_15676630631260 (sparse_attention).

reference:
  q = x @ Wq.T + bq ; k = x @ Wk.T + bk ; v = x @ Wv.T + bv        (per batch)
  scores = sigmoid(q @ k.T / sqrt(P))                               [B,S,S]
  out[b,i,j,:] = tril(i,j) * scores[b,i,j] * v[b,j,:]               [B,S,S,P]

B=2, S=512, D=256, P=128.  Output is 256 MB; the causal mask zeroes the
j>i region.  run_bass_kernel_spmd pre-zeroes ExternalOutput buffers
(donated zero buffers under PJRT), so the kernel only writes the j<=i
region — at 128-column tile granularity per row: row i writes j-tiles
0..i//128 (the partial diagonal tile is zeroed exactly via a
host-supplied mask).

Sharding (8 cores, one NEFF, SPMD): core c -> batch b=c//4, quarter
k=c%4.  Rows are assigned as 16-row blocks paired (m, 31-m) so every
core's multiset of per-row written-tile-counts is {1,1,2,2,3,3,4,4} per
block pair -> identical instruction stream on every core, only input
data differs, and every core writes exactly 20 MB of the 32 MB shard.

Per-core device program:
  K^T[p,s], Q^T[p,i] with per-partition bias via K=1 matmul; V[s,p]
  tiles.  scores^T[j,i] = sigmoid((K^T_tile)^T @ Q^T / sqrt(P)) *
  mask.  Output rows are produced as [j_partition, (jt, i, p)] slabs:
  broadcast row-scaling of V by score columns, batched 8 rows per DVE
  tensor_tensor (stride-0 broadcast APs) with a slice of rows done as
  per-row activation-scale ops on ACT to balance the two engines; then
  batched HWDGE DMAs ([j, jt, (i p)] — 4 KB contiguous runs per
  partition) into the [j, i_local, p]-layout local output.
"""

import os
import sys

import numpy as np

for _p in ("/root/.axon_site/_ro/trn_rl_repo", "/opt/trn_rl_repo"):
    if _p not in sys.path and os.path.isdir(_p):
        sys.path.append(_p)

import concourse.bass as bass
import concourse.mybir as mybir
from concourse.tile import TileContext
from concourse import bass_utils

F32 = mybir.dt.float32
B, S, D, P = 2, 512, 256, 128
NCORES = 8
GROUP = 8           # output rows per DMA group
NGROUPS = 128 // GROUP
INV_SQRT_P = float(1.0 / np.sqrt(np.float32(P)))
# (g, jt) group-tiles assigned to ACT instead of DVE (as 8 per-row ops):
# tune for DVE/ACT balance.
ACT_EVERY = 4  # every 4th group-tile goes to ACT -> 10/40


def _blocks16(k: int) -> list[int]:
    # 16-row blocks (32 per batch) for quarter k, ordered so written
    # j-tile count ti=m//8 ascends: [0,0,1,1,2,2,3,3]
    return [k, k + 4, k + 8, k + 12, 19 - k, 23 - k, 27 - k, 31 - k]


def _rows_sel(k: int) -> np.ndarray:
    return np.concatenate([np.arange(16 * m, 16 * m + 16) for m in _blocks16(k)])


def _build_nc() -> bass.Bass:
    nc = bass.Bass(trn_type="TRN2")

    xt = nc.dram_tensor("xt", [D, S], F32, kind="ExternalInput")     # x[b].T
    xq = nc.dram_tensor("xq", [D, 128], F32, kind="ExternalInput")   # x[b].T[:, rows]
    w3 = nc.dram_tensor("w3", [D, 3 * P], F32, kind="ExternalInput")  # [Wq|Wk|Wv].T
    b3 = nc.dram_tensor("b3", [1, 3 * P], F32, kind="ExternalInput")  # [bq|bk|bv]
    mk = nc.dram_tensor("mk", [4, 128, 128], F32, kind="ExternalInput")
    # local output layout [j, i_local, p]: per-DMA-partition runs are
    # (i,p)-contiguous (4 KB per 8-row group) instead of 512 B
    out = nc.dram_tensor("out", [S, 128, P], F32, kind="ExternalOutput")

    with TileContext(nc) as tc:
        with (
            tc.tile_pool(name="const", bufs=1) as cpool,
            tc.tile_pool(name="psA", bufs=1, space="PSUM") as psA,
            tc.tile_pool(name="psB", bufs=2, space="PSUM") as psB,
            tc.tile_pool(name="slab", bufs=3) as spool,
        ):
            xt_sb = cpool.tile([128, 2 * S], F32, tag="xt")
            nc.sync.dma_start(
                xt_sb[:].rearrange("q (c s) -> q c s", c=2),
                xt.rearrange("(c p) s -> p c s", p=128),
            )
            w_sb = cpool.tile([128, 2 * 3 * P], F32, tag="w3")  # [d_lo|d_hi]x[q|k|v]
            nc.sync.dma_start(
                w_sb[:].rearrange("q (c m) -> q c m", c=2),
                w3.rearrange("(c p) m -> p c m", p=128),
            )
            xq_sb = cpool.tile([128, 2 * 128], F32, tag="xq")
            nc.sync.dma_start(
                xq_sb[:].rearrange("q (c m) -> q c m", c=2),
                xq.rearrange("(c p) m -> p c m", p=128),
            )
            b_sb = cpool.tile([1, 3 * P], F32, tag="b3")
            nc.sync.dma_start(b_sb[:], b3[:])
            mk_sb = cpool.tile([128, 4 * 128], F32, tag="mk")
            nc.sync.dma_start(
                mk_sb[:].rearrange("q (t i) -> q t i", t=4),
                mk.rearrange("t j i -> j t i"),
            )
            bq_r = b_sb[0:1, 0:P]
            bk_r = b_sb[0:1, P : 2 * P]
            bv_r = b_sb[0:1, 2 * P : 3 * P]

            def wchunk(which: int, c: int):  # which: 0=q,1=k,2=v ; c: d-chunk
                return w_sb[:, (c * 3 + which) * P : (c * 3 + which + 1) * P]

            ones_sb = cpool.tile([1, S], F32, tag="ones")
            nc.vector.memset(ones_sb[:], 1.0)

            # K^T [p, s] = Wk @ x.T + bk  (bias per partition via K=1 matmul)
            kt_ps = psA.tile([128, S], F32, tag="ktps")
            nc.tensor.matmul(kt_ps[:], wchunk(1, 0), xt_sb[:, 0:S], start=True, stop=False)
            nc.tensor.matmul(kt_ps[:], wchunk(1, 1), xt_sb[:, S : 2 * S], start=False, stop=False)
            nc.tensor.matmul(kt_ps[:], bk_r, ones_sb[0:1, 0:S], start=False, stop=True)
            kt_t = []
            for jt in range(4):
                ktile = cpool.tile([128, 128], F32, tag=f"kt{jt}", name=f"kt{jt}")
                nc.scalar.copy(ktile[:], kt_ps[:, jt * 128 : (jt + 1) * 128])
                kt_t.append(ktile)

            # Q^T [p, i] over this core's 128 rows
            qt_ps = psA.tile([128, 128], F32, tag="qtps")
            nc.tensor.matmul(qt_ps[:], wchunk(0, 0), xq_sb[:, 0:128], start=True, stop=False)
            nc.tensor.matmul(qt_ps[:], wchunk(0, 1), xq_sb[:, 128:256], start=False, stop=False)
            nc.tensor.matmul(qt_ps[:], bq_r, ones_sb[0:1, 0:128], start=False, stop=True)
            qt_sb = cpool.tile([128, 128], F32, tag="qt")
            nc.scalar.copy(qt_sb[:], qt_ps[:])

            v_t = [None] * 4
            stm_t = [None] * 4

            def make_vt(jt: int):
                v_ps = psB.tile([128, P], F32, tag="vps", name=f"vps{jt}")
                nc.tensor.matmul(
                    v_ps[:], xt_sb[:, jt * 128 : (jt + 1) * 128], wchunk(2, 0),
                    start=True, stop=False,
                )
                nc.tensor.matmul(
                    v_ps[:], xt_sb[:, S + jt * 128 : S + (jt + 1) * 128], wchunk(2, 1),
                    start=False, stop=False,
                )
                nc.tensor.matmul(
                    v_ps[:], ones_sb[0:1, 0:128], bv_r, start=False, stop=True
                )
                vt = cpool.tile([128, P], F32, tag=f"v{jt}", name=f"v{jt}")
                nc.scalar.copy(vt[:], v_ps[:])
                v_t[jt] = vt

            def make_st(jt: int):
                s_ps = psB.tile([128, 128], F32, tag="sps", name=f"sps{jt}")
                nc.tensor.matmul(s_ps[:], kt_t[jt][:], qt_sb[:], start=True, stop=True)
                st = cpool.tile([128, 128], F32, tag=f"st{jt}", name=f"st{jt}")
                nc.scalar.activation(
                    st[:], s_ps[:], mybir.ActivationFunctionType.Sigmoid,
                    scale=INV_SQRT_P,
                )
                stm = cpool.tile([128, 128], F32, tag=f"stm{jt}", name=f"stm{jt}")
                nc.vector.tensor_mul(
                    stm[:], st[:], mk_sb[:, jt * 128 : (jt + 1) * 128]
                )
                stm_t[jt] = stm

            # Output slabs: groups of GROUP rows; class t = g//4 writes
            # j-tiles 0..t.  slab free layout (jt, i, p); DMA target is
            # out[j, i, p] -> [j_part, t, (i p)] with 4 KB runs.
            out_r = out.rearrange("(t j) i p -> j t (i p)", j=128)  # [128,4,16384]
            gt_idx = 0
            for g in range(NGROUPS):
                t_cls = g // (NGROUPS // 4)
                L = t_cls + 1
                if g % (NGROUPS // 4) == 0:
                    # entering class t: materialize V/scores tile t
                    make_vt(t_cls)
                    make_st(t_cls)
                slab = spool.tile(
                    [128, L * GROUP * 128], F32, tag=f"slab{L}", name=f"slab_g{g}"
                )
                for jt in range(L):
                    dst3 = slab[
                        :, jt * GROUP * 128 : (jt + 1) * GROUP * 128
                    ].rearrange("q (i p) -> q i p", i=GROUP)
                    if gt_idx % ACT_EVERY == ACT_EVERY - 1:
                        for ii in range(GROUP):
                            li = g * GROUP + ii
                            nc.scalar.mul(
                                dst3[:, ii, :],
                                v_t[jt][:],
                                mul=stm_t[jt][:, li : li + 1],
                            )
                    else:
                        v3 = v_t[jt][:].unsqueeze(1).broadcast_to([128, GROUP, 128])
                        s3 = (
                            stm_t[jt][:, g * GROUP : (g + 1) * GROUP]
                            .unsqueeze(2)
                            .broadcast_to([128, GROUP, 128])
                        )
                        nc.vector.tensor_mul(dst3, v3, s3)
                    gt_idx += 1
                nc.sync.dma_start(
                    out_r[:, 0:L, GROUP * 128 * g : GROUP * 128 * (g + 1)],
                    slab[:].rearrange("q (t ip) -> q t ip", t=L),
                )

    _split_multi_waits(nc)
    return nc


def _split_multi_waits(nc):
    """This toolchain's walrus accepts at most one sync wait per
    instruction; split extras into single-wait NoOps just before the
    instruction on the same engine queue (waits are ANDed preconditions,
    executed in order on the engine's queue — semantically identical)."""
    for fn in nc.m.functions:
        for blk in fn.blocks:
            insts = blk.instructions
            i = 0
            while i < len(insts):
                inst = insts[i]
                si = getattr(inst, "sync_info", None)
                if si is not None and si.on_wait is not None and len(si.on_wait) > 1:
                    waits = list(si.on_wait)
                    nops = [
                        mybir.InstNoOp(
                            name=nc.get_next_instruction_name(),
                            engine=inst.engine,
                            sync_info=mybir.SyncInfo(on_wait=[w], on_update=[]),
                            bass_nofuse=True,
                        )
                        for w in waits[:-1]
                    ]
                    si.on_wait = [waits[-1]]
                    insts[i:i] = nops
                    i += len(nops)
                i += 1


_NC_CACHE = None


def _get_nc():
    global _NC_CACHE
    if _NC_CACHE is None:
        _NC_CACHE = _build_nc()
    return _NC_CACHE


def _in_maps(x_set, Wq, bq, Wk, bk, Wv, bv):
    w3 = np.ascontiguousarray(
        np.concatenate([Wq.T, Wk.T, Wv.T], axis=1)
    ).astype(np.float32, copy=False)
    b3 = np.ascontiguousarray(np.concatenate([bq, bk, bv])[None, :]).astype(np.float32, copy=False)
    xts = [
        np.ascontiguousarray(x_set[b].T).astype(np.float32, copy=False)
        for b in range(B)
    ]
    jj = np.arange(128)
    maps = []
    for c in range(NCORES):
        b, k = divmod(c, 4)
        rows = _rows_sel(k)
        mask = np.empty((4, 128, 128), np.float32)
        for jt in range(4):
            mask[jt] = ((jt * 128 + jj)[:, None] <= rows[None, :]).astype(np.float32)
        maps.append(
            {
                "xt": xts[b],
                "xq": np.ascontiguousarray(xts[b][:, rows]),
                "w3": w3,
                "b3": b3,
                "mk": mask,
            }
        )
    return maps


def run(x_set, Wq, bq, Wk, bk, Wv, bv, **spmd_kwargs):
    nc = _get_nc()
    in_maps = _in_maps(x_set, Wq, bq, Wk, bk, Wv, bv)
    res = bass_utils.run_bass_kernel_spmd(
        nc, in_maps, core_ids=list(range(NCORES)), **spmd_kwargs
    )
    full = np.zeros((B, S, S, P), np.float32)
    for c in range(NCORES):
        b, k = divmod(c, 4)
        # core output is [j, i_local, p] -> scatter as [i_local, j, p]
        full[b, _rows_sel(k)] = res.results[c]["out"].transpose(1, 0, 2)
    return full, res


def kernel(x_set, Wq, bq, Wk, bk, Wv, bv):
    full, _ = run(x_set, Wq, bq, Wk, bk, Wv, bv)
    return full
